# revision 4
# baseline (speedup 1.0000x reference)
"""FlowNetC correlation layer on 8 Trainium2 NeuronCores.

Math: out[b, d, y, x] = (1/256) * sum_c in1[b,c,y,x] * in2pad[b,c,y+dy,x+dx]
with (dy, dx) on a 21x21 stride-2 grid spanning [-20, 20], zero padding 20.

Strategy (per core = one batch sample; batch is exactly 8):
- Displacements have stride 2, so the problem splits into 4 independent parity
  classes. Each class: in1c [256, 32, 48] against a padded in2c [256, 52, 68]
  with stride-1 displacements dy', dx' in [0, 20].
- Gram band matmuls: for each class and group of 4 subsampled x-columns, run 4
  concurrent col-tiled matmuls (M=32 each, tile_position=(0, 32*xg)). Tile xg's
  stationary is in1c[:, :, x0] (32 ys columns); its moving tensor is the 21-wide
  window in2c[:, :, x0:x0+21] over all 52 rows (N = 52*21 = 1092, split into
  PSUM-bank chunks of 504/504/84). PSUM partition p = 32*xg + ys then holds the
  full 441-displacement vector contiguously at columns [21*ys, 21*ys + 441).
- The per-partition shear (21*ys) is undone by 32 per-ys SBUF->SBUF DMAs per
  class-half (partition-exact stride on xg, in-partition strides on xsg/d).
- TensorE transposes flip dense [pixel, d] tiles to [d, pixel], evictions
  scatter into a d-major bf16 assembly buffer, and 4 output DMAs (gpsimd,
  bf16->f32 cast) write [441, 64, 96] with 24 KB contiguous runs per d.
- Matmul inputs are bf16 (1 cycle/column on the PE vs 4 for fp32); the 1/256
  normalization is folded into in1's bf16 cast exactly (exponent shift).
"""

import os
import sys

for _p in ("/opt/trn_rl_repo", "/root/.axon_site/_ro/trn_rl_repo"):
    if os.path.isdir(_p) and _p not in sys.path:
        sys.path.insert(0, _p)

from contextlib import ExitStack

import ml_dtypes
import numpy as np

import concourse.bacc as bacc
import concourse.bass as bass
import concourse.mybir as mybir
import concourse.tile as tile
from concourse.bass_utils import run_bass_kernel_spmd
from concourse.masks import make_identity

B, C, H, W = 8, 256, 64, 96
NYS, NXS = 32, 48          # subsampled class grid
RB, CB = 52, 68            # padded class grid (rows/cols)
ND = 441                   # displacements
WB = 1092                  # band width per xs-column (52 rows * 21 dx)
NG = 6                     # xs-columns per band (class half)
FB = NG * WB               # band free size
NPIX = H * W               # 6144
DCHUNKS = [(0, 128), (128, 128), (256, 128), (384, 57)]
GRAM_CHUNKS = [(0, 24), (24, 48), (48, 52)]  # ysB row ranges per PSUM bank

F32 = mybir.dt.float32
BF16 = mybir.dt.bfloat16


def build():
    nc = bacc.Bacc("TRN2", target_bir_lowering=False, debug=False, num_devices=8)
    in1p = nc.declare_dram_parameter("in1p", [2, 128, 4, NXS, NYS], BF16, isOutput=False)
    in2p = nc.declare_dram_parameter("in2p", [4, 2, 128, RB, CB], BF16, isOutput=False)
    outp = nc.declare_dram_parameter("out", [ND, H, W], F32, isOutput=True)

    with tile.TileContext(nc) as tc:
        with ExitStack() as ctx:
            const_pool = ctx.enter_context(tc.tile_pool(name="const", bufs=1))
            in2_pool = ctx.enter_context(tc.tile_pool(name="in2", bufs=2))
            band_pool = ctx.enter_context(tc.tile_pool(name="band", bufs=2))
            dense_pool = ctx.enter_context(tc.tile_pool(name="dense", bufs=2))
            out_pool = ctx.enter_context(tc.tile_pool(name="outsb", bufs=1))
            pg_pool = ctx.enter_context(tc.tile_pool(name="pg", bufs=2, space="PSUM"))
            pt_pool = ctx.enter_context(tc.tile_pool(name="pt", bufs=2, space="PSUM"))

            ident = const_pool.tile([128, 128], F32)
            make_identity(nc, ident)

            # resident in1: [c, k, cls, xs, ys]
            in1_sb = const_pool.tile([128, 2, 4, NXS, NYS], BF16)
            nc.sync.dma_start(
                out=bass.AP(in1_sb.tensor, in1_sb.offset,
                            [[2 * 4 * NXS * NYS, 128], [4 * NXS * NYS, 2], [1, 4 * NXS * NYS]]),
                in_=bass.AP(in1p, 0,
                            [[4 * NXS * NYS, 128], [128 * 4 * NXS * NYS, 2], [1, 4 * NXS * NYS]]),
            )

            # persistent d-major assembly buffers, one per d-chunk
            out_sb = [out_pool.tile([128, NPIX], BF16, tag=f"out{dc}", name=f"out_sb{dc}")
                      for dc in range(4)]

            ev_flip = 0
            for cid in range(4):
                py, px = cid // 2, cid % 2
                in2_sb = in2_pool.tile([128, 2, RB, CB], BF16)
                nc.sync.dma_start(
                    out=bass.AP(in2_sb.tensor, in2_sb.offset,
                                [[2 * RB * CB, 128], [RB * CB, 2], [1, RB * CB]]),
                    in_=bass.AP(in2p, cid * 2 * 128 * RB * CB,
                                [[RB * CB, 128], [128 * RB * CB, 2], [1, RB * CB]]),
                )
                for half in range(2):
                    band = band_pool.tile([128, FB], F32)
                    for g in range(NG):
                        xsg = half * NG + g
                        pg = pg_pool.tile([128, 3, 512], F32)
                        for k in range(2):
                            for xg in range(4):
                                x0 = 4 * xsg + xg
                                lhsT = in1_sb[:, k, cid, x0, :]
                                for ch, (r0, r1) in enumerate(GRAM_CHUNKS):
                                    ncols = (r1 - r0) * 21
                                    rhs = in2_sb[:, k, r0:r1, x0:x0 + 21]
                                    nc.tensor.matmul(
                                        pg[32 * xg:32 * (xg + 1), ch, 0:ncols],
                                        lhsT, rhs,
                                        start=(k == 0), stop=(k == 1),
                                        tile_position=(0, 32 * xg),
                                    )
                        # evict psum band chunks into packed band columns
                        for ch, (r0, r1) in enumerate(GRAM_CHUNKS):
                            ncols = (r1 - r0) * 21
                            dst = band[:, g * WB + r0 * 21: g * WB + r0 * 21 + ncols]
                            if ev_flip % 2 == 0:
                                nc.vector.tensor_copy(out=dst, in_=pg[:, ch, 0:ncols])
                            else:
                                nc.scalar.copy(out=dst, in_=pg[:, ch, 0:ncols])
                            ev_flip += 1

                    # de-shear: per-ys batched DMAs, band -> dense [pixel, d]
                    dense = dense_pool.tile([128, NG, ND], F32)
                    for ys in range(NYS):
                        src = bass.AP(band.tensor, band.offset + ys * FB + 21 * ys,
                                      [[32 * FB, 4], [WB, NG], [1, ND]])
                        dst = bass.AP(dense.tensor, dense.offset + ys * (NG * ND),
                                      [[32 * NG * ND, 4], [ND, NG], [1, ND]])
                        nc.sync.dma_start(out=dst, in_=src)

                    # transpose to d-major and scatter into assembly buffers
                    for dc, (d0, dcw) in enumerate(DCHUNKS):
                        for s in range(2):
                            pt = pt_pool.tile([128, 384], F32)
                            for j in range(3):
                                nc.tensor.transpose(
                                    pt[0:dcw, j * 128:(j + 1) * 128],
                                    dense[:, s * 3 + j, d0:d0 + dcw],
                                    ident[:],
                                )
                            ob = out_sb[dc]
                            src = bass.AP(pt.tensor, pt.offset,
                                          [[384, dcw], [128, 3], [32, 4], [1, 32]])
                            doff = 96 * py + px + 8 * (half * NG + 3 * s)
                            dst = bass.AP(ob.tensor, ob.offset + doff,
                                          [[NPIX, dcw], [8, 3], [2, 4], [192, 32]])
                            if ev_flip % 2 == 0:
                                nc.vector.tensor_copy(out=dst, in_=src)
                            else:
                                nc.scalar.copy(out=dst, in_=src)
                            ev_flip += 1

            # output: one cast DMA per d-chunk, 24KB contiguous runs per d
            for dc, (d0, dcw) in enumerate(DCHUNKS):
                ob = out_sb[dc]
                nc.gpsimd.dma_start(
                    out=bass.AP(outp, d0 * NPIX, [[NPIX, dcw], [1, NPIX]]),
                    in_=bass.AP(ob.tensor, ob.offset, [[NPIX, dcw], [1, NPIX]]),
                )

    nc.compile()
    return nc


def prep_inputs(input1, input2):
    """Host-side: parity split, pad, bf16 cast, fold 1/256 into in1."""
    in_maps = []
    for b in range(B):
        a1 = (input1[b].astype(np.float32) / 256.0).reshape(2, 128, H, W)
        a2 = input2[b].astype(np.float32).reshape(2, 128, H, W)
        in1p = np.empty((2, 128, 4, NXS, NYS), dtype=ml_dtypes.bfloat16)
        in2p = np.zeros((4, 2, 128, RB, CB), dtype=ml_dtypes.bfloat16)
        for cid in range(4):
            py, px = cid // 2, cid % 2
            in1p[:, :, cid] = a1[:, :, py::2, px::2].transpose(0, 1, 3, 2).astype(ml_dtypes.bfloat16)
            in2p[cid, :, :, 10:42, 10:58] = a2[:, :, py::2, px::2].astype(ml_dtypes.bfloat16)
        in_maps.append({"in1p": in1p, "in2p": in2p})
    return in_maps


_NC = None


def get_nc():
    global _NC
    if _NC is None:
        _NC = build()
    return _NC


def kernel(input1, input2):
    nc = get_nc()
    in_maps = prep_inputs(np.asarray(input1), np.asarray(input2))
    r = run_bass_kernel_spmd(nc, in_maps, core_ids=list(range(8)))
    return np.stack([r.results[i]["out"] for i in range(B)]).astype(np.float32)
